# revision 4
# baseline (speedup 1.0000x reference)
"""Distributed Trainium2 kernel for the additive-attention alignment predictor.

Math: score[b,t,u] = sum_h w_h * tanh(ep[b,t,h] + dp[b,u,h]);  out = softmax_u(score)
  where ep = enc @ W_enc (bias folded into dp), dp = dec @ W_dec + b_enc + b_dec.
  (b_score dropped: softmax is shift-invariant; pure-x terms likewise dropped.)

tanh(z) on |z|<=6 is replaced by the separable expansion
  tanh(z) ~= c1*z + c3*z^3 + sum_f cf sin(w_f z)
with 5 frequencies built from 2 half-angle base evaluations (f1, f2) plus a
double-angle ladder (2*f1, 2*f2, 4*f2).  sin(w(x+y)) = sin cos + cos sin and
the cubic expand into 13 rank-1 products, so the whole [T,U,H] contraction
becomes TensorEngine matmuls.  Sin planes are stored as sin/2^g; the 2^g
factors fold into the score coefficients, which are pre-broadcast on the host
into a [P, slot, m, U] tile (w_bc) so each B-plane build is one big
TENSOR_TENSOR instead of many small per-(plane,m) tensor_scalars.

Engine split: ACT does only the 4 base Sin evals (+ final Exp); all squares
run on GpSimd; DVE does the half/double-angle combines, B planes and softmax.
Dummy ones-matmuls at kernel start keep the PE HAM clock at 2.4 GHz through
the input DMA wait.

Sharding: data-parallel over (B, T/2): core c handles batch c//2, t-half c%2.
No cross-core communication.
"""

import math

import numpy as np
import ml_dtypes

import concourse.bass as bass
import concourse.tile as tile
from concourse import bacc, mybir
from concourse.bass_utils import run_bass_kernel_spmd

# Problem shapes (hardcoded per spec)
B, T, U = 4, 800, 150
D, H = 512, 256
NCORES = 8
TPC = T * B // NCORES  # 400 t-rows per core
P = 128
KT = D // P
HT = H // P
W550 = TPC + U  # 550
TBLK = [(i * P, min(P, TPC - i * P)) for i in range((TPC + P - 1) // P)]
N_WARMUP = 18

# Fitted expansion (fit_tanh2.py config C):
#   tanh(z) ~= C1 z + C3 z^3 + sum cf sin(w z),  freqs {f1, f2, 2f1, 2f2, 4f2}
F1, F2 = 1.2150000011920929, 0.9102464900903208
FREQS = [F1, F2, 2 * F1, 2 * F2, 4 * F2]
C1, C3 = 0.40834739, -0.00686156
CF = [0.16673546, 0.1569309, 0.03624548, 0.06357671, 0.01070035]
# generation g: stored sin plane is sin(w z)/2^g
GENS = [1, 1, 2, 2, 3]

F32 = mybir.dt.float32
BF16 = mybir.dt.bfloat16
AF = mybir.ActivationFunctionType
ALU = mybir.AluOpType

# w_bc slots: 5 freq slots (cf * 2^g * w), then 3c3*w, then c1*w
NSLOT = 7


def _build_graph():
    nc = bacc.Bacc()
    enc_x = nc.declare_dram_parameter("enc_t", [D, TPC], BF16, isOutput=False)
    dec_x = nc.declare_dram_parameter("dec_t", [D, U], BF16, isOutput=False)
    wcat_x = nc.declare_dram_parameter("wcat", [D, 2 * H], BF16, isOutput=False)
    wbc_x = nc.declare_dram_parameter("w_bc", [P, NSLOT, HT, U], BF16, isOutput=False)
    bias_x = nc.declare_dram_parameter("bias2", [P, HT], F32, isOutput=False)
    out_x = nc.declare_dram_parameter("out", [TPC, U], F32, isOutput=True)

    enc_v = enc_x[:].rearrange("(k p) t -> p k t", p=P)
    dec_v = dec_x[:].rearrange("(k p) u -> p k u", p=P)
    wcat_v = wcat_x[:].rearrange("(k p) h -> p k h", p=P)

    with tile.TileContext(nc) as tc:
        with (
            tc.tile_pool(name="const", bufs=1) as const,
            tc.tile_pool(name="soft", bufs=1) as soft,
            tc.tile_pool(name="ppsum", bufs=1, space="PSUM") as ppsum,
            tc.tile_pool(name="spsum", bufs=1, space="PSUM") as spsum,
        ):
            # ---- constants
            ones_a = const.tile([P, P], BF16)
            nc.vector.memset(ones_a, 1.0)
            halfpi = const.tile([P, 1], F32)
            nc.vector.memset(halfpi, math.pi / 2)

            # ---- input DMAs: few large descriptors on separate queues
            enc_sb = const.tile([P, KT, TPC], BF16)
            dec_sb = const.tile([P, KT, U], BF16)
            wcat_sb = const.tile([P, KT, 2 * H], BF16)
            wbc_sb = const.tile([P, NSLOT, HT, U], BF16)
            bias_sb = const.tile([P, HT], F32)
            nc.gpsimd.dma_start(out=wcat_sb[:, 0:2, :], in_=wcat_v[:, 0:2, :])
            nc.sync.dma_start(out=enc_sb[:, 0:2, :], in_=enc_v[:, 0:2, :])
            nc.sync.dma_start(out=enc_sb[:, 2:4, :], in_=enc_v[:, 2:4, :])
            nc.scalar.dma_start(out=dec_sb, in_=dec_v[:, :, :])
            nc.gpsimd.dma_start(out=wcat_sb[:, 2:4, :], in_=wcat_v[:, 2:4, :])
            nc.scalar.dma_start(out=bias_sb, in_=bias_x[:])
            nc.sync.dma_start(out=wbc_sb, in_=wbc_x[:])

            # ---- PE warm-up: keep HAM busy through the DMA wait
            ps_ep = [ppsum.tile([P, TPC], F32, name=f"ps_ep{m}") for m in range(HT)]
            ps_dp = [ppsum.tile([P, U], F32, name=f"ps_dp{m}") for m in range(HT)]
            for i in range(N_WARMUP):
                nc.tensor.matmul(
                    ps_dp[0][:, 0:P], lhsT=ones_a, rhs=ones_a,
                    start=True, stop=True, skip_group_check=True,
                )

            # ---- projections: ep[m] = sum_k we_k^T enc_k ; dp likewise
            for kk in (0, 1):  # DMA halves
                for k in (2 * kk, 2 * kk + 1):
                    for m in range(HT):
                        nc.tensor.matmul(
                            ps_ep[m],
                            lhsT=wcat_sb[:, k, m * P:(m + 1) * P],
                            rhs=enc_sb[:, k, :],
                            start=(k == 0), stop=(k == KT - 1),
                        )
                        nc.tensor.matmul(
                            ps_dp[m],
                            lhsT=wcat_sb[:, k, 2 * H - (2 - m) * P: 2 * H - (1 - m) * P],
                            rhs=dec_sb[:, k, :],
                            start=(k == 0), stop=(k == KT - 1),
                        )

            # ---- epdp [P, m, 550]: x part 0:400, y part 400:550 (bf16)
            epdp = const.tile([P, HT, W550], BF16)
            for m in range(HT):
                nc.vector.tensor_copy(epdp[:, m, 0:TPC], ps_ep[m])
                nc.vector.tensor_scalar_add(
                    out=epdp[:, m, TPC:W550], in0=ps_dp[m],
                    scalar1=bias_sb[:, m:m + 1],
                )

            # ---- ACT: 4 base Sin evals into [P, f, m, 550] tiles
            sh_all = const.tile([P, 2, HT, W550], BF16)
            ch_all = const.tile([P, 2, HT, W550], BF16)
            nc.scalar.activation(out=sh_all[:, 0], in_=epdp, func=AF.Sin, scale=float(F1 / 2))
            nc.scalar.activation(out=ch_all[:, 0], in_=epdp, func=AF.Sin, scale=float(F1 / 2), bias=halfpi[:, :])
            nc.scalar.activation(out=sh_all[:, 1], in_=epdp, func=AF.Sin, scale=float(F2 / 2))
            nc.scalar.activation(out=ch_all[:, 1], in_=epdp, func=AF.Sin, scale=float(F2 / 2), bias=halfpi[:, :])

            # ---- GpSimd: squares
            sq = const.tile([P, HT, W550], BF16)          # epdp^2
            nc.gpsimd.tensor_tensor(out=sq, in0=epdp, in1=epdp, op=ALU.mult)
            shsq = const.tile([P, 2, HT, W550], BF16)     # sh^2 per base freq
            nc.gpsimd.tensor_tensor(out=shsq[:, 0], in0=sh_all[:, 0], in1=sh_all[:, 0], op=ALU.mult)

            # ---- early poly B planes (y parts, strided [m,150] APs)
            ep_y = epdp[:, :, TPC:W550]
            sq_y = sq[:, :, TPC:W550]
            v3 = const.tile([P, HT, U], BF16)
            nc.vector.tensor_tensor(out=v3, in0=ep_y, in1=sq_y, op=ALU.mult)
            b_x2 = const.tile([P, HT, U], BF16)   # (3c3 w) y
            nc.vector.tensor_tensor(out=b_x2, in0=ep_y, in1=wbc_sb[:, 5], op=ALU.mult)
            b_x = const.tile([P, HT, U], BF16)    # (y^2 + c1/(3c3)) (3c3 w)
            nc.vector.scalar_tensor_tensor(
                out=b_x, in0=sq_y, scalar=float(C1 / (3 * C3)), in1=wbc_sb[:, 5],
                op0=ALU.add, op1=ALU.mult,
            )
            ytild = const.tile([P, HT, U], BF16)  # y + (c3/c1) y^3
            nc.vector.scalar_tensor_tensor(
                out=ytild, in0=v3, scalar=float(C3 / C1), in1=ep_y,
                op0=ALU.mult, op1=ALU.add,
            )
            b_lin = const.tile([P, HT, U], BF16)  # (c1 w)(y + (c3/c1) y^3)
            nc.vector.tensor_tensor(out=b_lin, in0=ytild, in1=wbc_sb[:, 6], op=ALU.mult)
            b_lin2 = const.tile([P, U], BF16)     # sum over m (ones-lhsT pair)
            nc.vector.tensor_tensor(out=b_lin2, in0=b_lin[:, 0], in1=b_lin[:, 1], op=ALU.add)

            # ---- psum accumulators: one bank per t-block
            sp = spsum.tile([P, len(TBLK), 512], F32)
            mm_i = [0] * len(TBLK)
            N_MM = 2 * 12 + 1  # per t-block: 12 m-paired entries + ones pair

            def emit_phase(entries, final=False):
                # entries: list of (lhsT_fn(m, sl), rhs_fn(m)) or (lhsT_fn, rhs, None) ones-style
                for tb, (t0, pn) in enumerate(TBLK):
                    sl = slice(t0, t0 + pn)
                    for ent in entries:
                        if ent[0] == "ones":
                            nc.tensor.matmul(
                                sp[:pn, tb, 0:U], lhsT=ones_a[:, :pn], rhs=ent[1],
                                start=(mm_i[tb] == 0), stop=(mm_i[tb] == N_MM - 1),
                            )
                            mm_i[tb] += 1
                        else:
                            a_fn, b_fn = ent
                            for m in range(HT):
                                nc.tensor.matmul(
                                    sp[:pn, tb, 0:U], lhsT=a_fn(m, sl), rhs=b_fn(m),
                                    start=(mm_i[tb] == 0), stop=(mm_i[tb] == N_MM - 1),
                                )
                                mm_i[tb] += 1
                    if final:
                        expt = soft.tile([P, U], F32, name=f"expt{tb}", bufs=2)
                        nc.scalar.activation(out=expt[:pn], in_=sp[:pn, tb, 0:U], func=AF.Exp, scale=1.0)
                        ssum = soft.tile([P, 1], F32, name=f"ssum{tb}", bufs=2)
                        nc.vector.tensor_reduce(out=ssum[:pn], in_=expt[:pn], axis=mybir.AxisListType.X, op=ALU.add)
                        nc.vector.reciprocal(out=ssum[:pn], in_=ssum[:pn])
                        outt = soft.tile([P, U], F32, name=f"outt{tb}", bufs=2)
                        nc.vector.tensor_scalar_mul(out=outt[:pn], in0=expt[:pn], scalar1=ssum[:pn])
                        nc.sync.dma_start(out=out_x[t0:t0 + pn, :], in_=outt[:pn])

            # phase 1: poly pairs (ready first)
            emit_phase([
                (lambda m, s: epdp[:, m, s], lambda m: b_x[:, m]),
                (lambda m, s: sq[:, m, s], lambda m: b_x2[:, m]),
                ("ones", b_lin2),
            ])

            # ---- f1 combines (DVE) + B planes, then f1 MMs
            s12 = const.tile([P, 2, HT, W550], BF16)   # sin(f)/2
            c12 = const.tile([P, 2, HT, W550], BF16)   # cos(f)
            b_s = []  # per freq: [P, 2(sin_y/cos_y), m, U]; rhs for (cos_x, *sin_y*) is b[0]
            for i in range(5):
                b_s.append(const.tile([P, 2, HT, U], BF16, name=f"b_f{i}"))

            def freq_combine(f):
                # sin = sh*ch, cos = 1 - 2 sh^2 (shsq computed on gpsimd)
                nc.vector.tensor_tensor(out=s12[:, f], in0=sh_all[:, f], in1=ch_all[:, f], op=ALU.mult)
                nc.vector.tensor_scalar(
                    out=c12[:, f], in0=shsq[:, f], scalar1=-2.0, scalar2=1.0,
                    op0=ALU.mult, op1=ALU.add,
                )
                nc.vector.tensor_tensor(out=b_s[f][:, 0], in0=s12[:, f, :, TPC:W550], in1=wbc_sb[:, f], op=ALU.mult)
                nc.vector.tensor_tensor(out=b_s[f][:, 1], in0=c12[:, f, :, TPC:W550], in1=wbc_sb[:, f], op=ALU.mult)

            def freq_entries(i, s_t, c_t):
                # pair (sin_x, w cf cos_y) + (cos_x, w cf sin_y)
                return [
                    (lambda m, s, t=s_t: t[:, m, s], lambda m, i=i: b_s[i][:, 1, m]),
                    (lambda m, s, t=c_t: t[:, m, s], lambda m, i=i: b_s[i][:, 0, m]),
                ]

            freq_combine(0)
            emit_phase(freq_entries(0, s12[:, 0], c12[:, 0]))

            # f2: shsq on gpsimd (after ACT sh2 done), then combine + MMs
            nc.gpsimd.tensor_tensor(out=shsq[:, 1], in0=sh_all[:, 1], in1=sh_all[:, 1], op=ALU.mult)
            freq_combine(1)
            emit_phase(freq_entries(1, s12[:, 1], c12[:, 1]))

            # ---- ladder gen1: F3=2f1, F4=2f2 (batched over both freqs)
            s34 = const.tile([P, 2, HT, W550], BF16)   # sin(2f)/4
            c34 = const.tile([P, 2, HT, W550], BF16)
            ssq34 = const.tile([P, 2, HT, W550], BF16)
            nc.vector.tensor_tensor(out=s34, in0=s12, in1=c12, op=ALU.mult)
            nc.gpsimd.tensor_tensor(out=ssq34, in0=s12, in1=s12, op=ALU.mult)
            nc.vector.tensor_scalar(
                out=c34, in0=ssq34, scalar1=-8.0, scalar2=1.0, op0=ALU.mult, op1=ALU.add,
            )
            nc.vector.tensor_tensor(out=b_s[2][:, 0], in0=s34[:, 0, :, TPC:W550], in1=wbc_sb[:, 2], op=ALU.mult)
            nc.vector.tensor_tensor(out=b_s[2][:, 1], in0=c34[:, 0, :, TPC:W550], in1=wbc_sb[:, 2], op=ALU.mult)
            nc.vector.tensor_tensor(out=b_s[3][:, 0], in0=s34[:, 1, :, TPC:W550], in1=wbc_sb[:, 3], op=ALU.mult)
            nc.vector.tensor_tensor(out=b_s[3][:, 1], in0=c34[:, 1, :, TPC:W550], in1=wbc_sb[:, 3], op=ALU.mult)
            emit_phase(freq_entries(2, s34[:, 0], c34[:, 0]) + freq_entries(3, s34[:, 1], c34[:, 1]))

            # ---- ladder gen2: F5=4f2; final phase + per-block softmax
            s5 = const.tile([P, HT, W550], BF16)       # sin(4f2)/8
            c5 = const.tile([P, HT, W550], BF16)
            ssq5 = const.tile([P, HT, W550], BF16)
            nc.vector.tensor_tensor(out=s5, in0=s34[:, 1], in1=c34[:, 1], op=ALU.mult)
            nc.gpsimd.tensor_tensor(out=ssq5, in0=s34[:, 1], in1=s34[:, 1], op=ALU.mult)
            nc.vector.tensor_scalar(
                out=c5, in0=ssq5, scalar1=-32.0, scalar2=1.0, op0=ALU.mult, op1=ALU.add,
            )
            nc.vector.tensor_tensor(out=b_s[4][:, 0], in0=s5[:, :, TPC:W550], in1=wbc_sb[:, 4], op=ALU.mult)
            nc.vector.tensor_tensor(out=b_s[4][:, 1], in0=c5[:, :, TPC:W550], in1=wbc_sb[:, 4], op=ALU.mult)
            emit_phase(freq_entries(4, s5, c5), final=True)

    nc.finalize()
    return nc


_NC_CACHE = None


def kernel(**inputs: np.ndarray) -> np.ndarray:
    global _NC_CACHE
    bfd = ml_dtypes.bfloat16
    enc = np.asarray(inputs["encoder_out"], dtype=np.float32)
    dec = np.asarray(inputs["decoder_out"], dtype=np.float32)
    w_enc = np.asarray(inputs["W_enc"], np.float32)
    b_enc = np.asarray(inputs["b_enc"], dtype=np.float32)
    w_dec = np.asarray(inputs["W_dec"], np.float32)
    b_dec = np.asarray(inputs["b_dec"], dtype=np.float32)
    w_score = np.asarray(inputs["w_score"], dtype=np.float32)
    # b_score dropped: softmax(x + c) == softmax(x)

    wcat = np.ascontiguousarray(
        np.concatenate([w_enc, w_dec], axis=1).astype(bfd)
    )
    bias2 = np.ascontiguousarray((b_enc + b_dec).reshape(HT, P).T)

    # w_bc[p, slot, m, u]: slots 0..4 freq coefs cf*2^g*w, 5: 3c3*w, 6: c1*w
    wm = w_score.reshape(HT, P).T  # [P, m]
    w_bc = np.empty((P, NSLOT, HT, U), dtype=np.float32)
    for i in range(5):
        w_bc[:, i] = (CF[i] * (2 ** GENS[i]) * wm)[:, :, None]
    w_bc[:, 5] = (3 * C3 * wm)[:, :, None]
    w_bc[:, 6] = (C1 * wm)[:, :, None]
    w_bc = np.ascontiguousarray(w_bc.astype(bfd))

    in_maps = []
    for c in range(NCORES):
        b = c // (NCORES // B)
        t0 = (c % (NCORES // B)) * TPC
        in_maps.append(
            {
                "enc_t": np.ascontiguousarray(enc[b, t0:t0 + TPC, :].T.astype(bfd)),
                "dec_t": np.ascontiguousarray(dec[b].T.astype(bfd)),
                "wcat": wcat,
                "w_bc": w_bc,
                "bias2": bias2,
            }
        )

    if _NC_CACHE is None:
        _NC_CACHE = _build_graph()
    res = run_bass_kernel_spmd(_NC_CACHE, in_maps, core_ids=list(range(NCORES)))

    out = np.empty((B, T, U), dtype=np.float32)
    for c in range(NCORES):
        b = c // (NCORES // B)
        t0 = (c % (NCORES // B)) * TPC
        out[b, t0:t0 + TPC, :] = res.results[c]["out"]
    return out


# revision 7
# speedup vs baseline: 1.2348x; 1.2348x over previous
"""Distributed Trainium2 kernel for the additive-attention alignment predictor.

Math: score[b,t,u] = sum_h w_h * tanh(ep[b,t,h] + dp[b,u,h]);  out = softmax_u(score)
  where ep = enc @ W_enc (bias folded into dp), dp = dec @ W_dec + b_enc + b_dec.
  (b_score dropped: softmax is shift-invariant; pure-x terms likewise dropped.)

tanh(z) on |z|<=6 is replaced by the separable expansion
  tanh(z) ~= c1*z + c3*z^3 + sum_f cf sin(w_f z)
with 5 frequencies built from 2 half-angle base evaluations (f1, f2) plus a
double-angle ladder (2*f1, 2*f2, 4*f2).  sin(w(x+y)) = sin cos + cos sin and
the cubic expand into 13 rank-1 products, so the whole [T,U,H] contraction
becomes TensorEngine matmuls.  Sin planes are stored as sin/2^g; the 2^g
factors fold into the score coefficients (w_bc tile, host-precomputed).

Layout/perf notes: all DRAM inputs are partition-major so each DMA packet is a
1.6-4KB contiguous run; tiles keep <=2 free dims (3+ free dims lose DVE perf
modes); the sin-plane squares run on ACT's idle tail (GpSimd TT measured
~3x slower than spec); dummy ones-matmuls at kernel start keep the PE HAM
clock warm through the input DMA wait.

Sharding: data-parallel over (B, T/2): core c handles batch c//2, t-half c%2.
No cross-core communication.
"""

import math

import numpy as np
import ml_dtypes

import concourse.bass as bass
import concourse.tile as tile
from concourse import bacc, mybir
from concourse.bass_utils import run_bass_kernel_spmd

# Problem shapes (hardcoded per spec)
B, T, U = 4, 800, 150
D, H = 512, 256
NCORES = 8
TPC = T * B // NCORES  # 400 t-rows per core
P = 128
KT = D // P
HT = H // P
W550 = TPC + U  # 550
TBLK = [(i * P, min(P, TPC - i * P)) for i in range((TPC + P - 1) // P)]
N_WARMUP = 16

# Fitted expansion (fit_tanh2.py config C, coefs refit for these exact freqs):
#   tanh(z) ~= C1 z + C3 z^3 + sum cf sin(w z),  freqs {f1, f2, 2f1, 2f2, 4f2}
F1, F2 = 1.2150000011920929, 0.9102464900903208
FREQS = [F1, F2, 2 * F1, 2 * F2, 4 * F2]
C1, C3 = 0.40834739, -0.00686156
CF = [0.16673546, 0.1569309, 0.03624548, 0.06357671, 0.01070035]
GENS = [1, 1, 2, 2, 3]  # stored sin plane is sin(w z)/2^g

F32 = mybir.dt.float32
BF16 = mybir.dt.bfloat16
AF = mybir.ActivationFunctionType
ALU = mybir.AluOpType

# w_bc slots: 5 freq slots (cf * 2^g * w), then 3c3*w, then c1*w, then c3*w
NSLOT = 8


def _build_graph():
    nc = bacc.Bacc()
    # partition-major inputs: [P, k-major free] so DMA runs are contiguous
    enc_x = nc.declare_dram_parameter("enc_pm", [P, KT * TPC], BF16, isOutput=False)
    dec_x = nc.declare_dram_parameter("dec_pm", [P, KT * U], BF16, isOutput=False)
    wcat_x = nc.declare_dram_parameter("wcat_pm", [P, KT * 2 * H], BF16, isOutput=False)
    wbc_x = nc.declare_dram_parameter("w_bc", [P, NSLOT * HT * U], BF16, isOutput=False)
    bias_x = nc.declare_dram_parameter("bias2", [P, HT], F32, isOutput=False)
    out_x = nc.declare_dram_parameter("out", [TPC, U], F32, isOutput=True)

    with tile.TileContext(nc) as tc:
        with (
            tc.tile_pool(name="const", bufs=1) as const,
            tc.tile_pool(name="soft", bufs=1) as soft,
            tc.tile_pool(name="ppsum", bufs=1, space="PSUM") as ppsum,
            tc.tile_pool(name="spsum", bufs=1, space="PSUM") as spsum,
        ):
            # ---- input DMAs first: few large contiguous descriptors
            enc_sb = const.tile([P, KT, TPC], BF16)
            dec_sb = const.tile([P, KT, U], BF16)
            wcat_sb = const.tile([P, KT, 2 * H], BF16)
            wbc_sb = const.tile([P, NSLOT, HT, U], BF16)
            bias_sb = const.tile([P, HT], F32)
            HW = KT * TPC // 2
            WW = KT * 2 * H // 2
            nc.sync.dma_start(out=enc_sb[:, 0:2, :], in_=enc_x[:, 0:HW])
            nc.gpsimd.dma_start(out=wcat_sb[:, 0:2, :], in_=wcat_x[:, 0:WW])
            nc.scalar.dma_start(out=dec_sb, in_=dec_x[:, :])
            nc.sync.dma_start(out=enc_sb[:, 2:4, :], in_=enc_x[:, HW:])
            nc.gpsimd.dma_start(out=wcat_sb[:, 2:4, :], in_=wcat_x[:, WW:])
            nc.scalar.dma_start(out=bias_sb, in_=bias_x[:])
            nc.sync.dma_start(out=wbc_sb, in_=wbc_x[:])

            # ---- constants
            ones_a = const.tile([P, P], BF16)
            nc.vector.memset(ones_a, 1.0)
            halfpi = const.tile([P, 1], F32)
            nc.vector.memset(halfpi, math.pi / 2)

            # ---- PE warm-up through the DMA wait
            ps_ep = [ppsum.tile([P, TPC], F32, name=f"ps_ep{m}") for m in range(HT)]
            ps_dp = [ppsum.tile([P, U], F32, name=f"ps_dp{m}") for m in range(HT)]
            for _ in range(N_WARMUP):
                nc.tensor.matmul(
                    ps_dp[0][:, 0:P], lhsT=ones_a, rhs=ones_a,
                    start=True, stop=True, skip_group_check=True,
                )

            # ---- projections
            for k in range(KT):
                for m in range(HT):
                    nc.tensor.matmul(
                        ps_ep[m],
                        lhsT=wcat_sb[:, k, m * P:(m + 1) * P],
                        rhs=enc_sb[:, k, :],
                        start=(k == 0), stop=(k == KT - 1),
                    )
                    nc.tensor.matmul(
                        ps_dp[m],
                        lhsT=wcat_sb[:, k, H + m * P:H + (m + 1) * P],
                        rhs=dec_sb[:, k, :],
                        start=(k == 0), stop=(k == KT - 1),
                    )

            # ---- epdp [P, m, 550]: x part 0:400, y part 400:550 (bf16)
            epdp = const.tile([P, HT, W550], BF16)
            for m in range(HT):
                nc.vector.tensor_copy(epdp[:, m, 0:TPC], ps_ep[m])
                nc.vector.tensor_scalar_add(
                    out=epdp[:, m, TPC:W550], in0=ps_dp[m],
                    scalar1=bias_sb[:, m:m + 1],
                )
            ep_y = epdp[:, :, TPC:W550]

            # ---- ACT: base Sin evals (then squares on ACT's tail)
            sh1 = const.tile([P, HT, W550], BF16)
            ch1 = const.tile([P, HT, W550], BF16)
            sh2 = const.tile([P, HT, W550], BF16)
            ch2 = const.tile([P, HT, W550], BF16)
            nc.scalar.activation(out=sh1, in_=epdp, func=AF.Sin, scale=float(F1 / 2))
            nc.scalar.activation(out=ch1, in_=epdp, func=AF.Sin, scale=float(F1 / 2), bias=halfpi[:, :])
            nc.scalar.activation(out=sh2, in_=epdp, func=AF.Sin, scale=float(F2 / 2))
            nc.scalar.activation(out=ch2, in_=epdp, func=AF.Sin, scale=float(F2 / 2), bias=halfpi[:, :])

            # plane tiles (all [P, HT, W550], <=2 free dims)
            s1 = const.tile([P, HT, W550], BF16)   # sin(f1)/2
            c1 = const.tile([P, HT, W550], BF16)   # cos(f1)
            s2 = const.tile([P, HT, W550], BF16)
            c2 = const.tile([P, HT, W550], BF16)
            s3 = const.tile([P, HT, W550], BF16)   # sin(2f1)/4
            c3_ = const.tile([P, HT, W550], BF16)
            s4 = const.tile([P, HT, W550], BF16)   # sin(2f2)/4
            c4 = const.tile([P, HT, W550], BF16)
            s5 = const.tile([P, HT, W550], BF16)   # sin(4f2)/8
            c5 = const.tile([P, HT, W550], BF16)
            shsq1 = const.tile([P, HT, W550], BF16)
            shsq2 = const.tile([P, HT, W550], BF16)
            t3 = const.tile([P, HT, W550], BF16)   # s1^2
            t4 = const.tile([P, HT, W550], BF16)   # s2^2
            t5 = const.tile([P, HT, W550], BF16)   # s4^2
            sq = const.tile([P, HT, W550], BF16)   # epdp^2

            # ---- DVE combine chain (program order = readiness order)
            nc.vector.tensor_tensor(out=shsq1, in0=sh1, in1=sh1, op=ALU.mult)
            nc.vector.tensor_tensor(out=s1, in0=sh1, in1=ch1, op=ALU.mult)
            nc.vector.tensor_scalar(out=c1, in0=shsq1, scalar1=-2.0, scalar2=1.0, op0=ALU.mult, op1=ALU.add)
            # s1^2 on ACT's idle tail (emitted after s1 so deps resolve)
            nc.scalar.activation(out=t3, in_=s1, func=AF.Square, scale=1.0)
            nc.vector.tensor_tensor(out=sq, in0=epdp, in1=epdp, op=ALU.mult)

            b_s = [const.tile([P, 2, HT, U], BF16, name=f"b_f{i}") for i in range(5)]

            def b_freq(i, s_t, c_t):
                # rhs planes: b_s[i][:,0] = w cf 2^g * sin_y ; [:,1] = ... * cos_y
                nc.vector.tensor_tensor(out=b_s[i][:, 0], in0=s_t[:, :, TPC:W550], in1=wbc_sb[:, i], op=ALU.mult)
                nc.vector.tensor_tensor(out=b_s[i][:, 1], in0=c_t[:, :, TPC:W550], in1=wbc_sb[:, i], op=ALU.mult)

            b_freq(0, s1, c1)

            # poly B planes
            sq_y = sq[:, :, TPC:W550]
            v3 = const.tile([P, HT, U], BF16)
            nc.vector.tensor_tensor(out=v3, in0=ep_y, in1=sq_y, op=ALU.mult)
            b_x2 = const.tile([P, HT, U], BF16)   # (3c3 w) y
            nc.vector.tensor_tensor(out=b_x2, in0=ep_y, in1=wbc_sb[:, 5], op=ALU.mult)
            b_x = const.tile([P, HT, U], BF16)    # (y^2 + c1/(3c3)) (3c3 w)
            nc.vector.scalar_tensor_tensor(
                out=b_x, in0=sq_y, scalar=float(C1 / (3 * C3)), in1=wbc_sb[:, 5],
                op0=ALU.add, op1=ALU.mult,
            )
            b_l1 = const.tile([P, HT, U], BF16)   # (c1 w) y
            nc.vector.tensor_tensor(out=b_l1, in0=ep_y, in1=wbc_sb[:, 6], op=ALU.mult)
            b_l3 = const.tile([P, HT, U], BF16)   # (c3 w) y^3
            nc.vector.tensor_tensor(out=b_l3, in0=v3, in1=wbc_sb[:, 7], op=ALU.mult)
            b_lin = const.tile([P, HT, U], BF16)
            nc.vector.tensor_tensor(out=b_lin, in0=b_l1, in1=b_l3, op=ALU.add)
            b_lin2 = const.tile([P, U], BF16)     # m-sum for the ones-lhsT pair
            nc.vector.tensor_tensor(out=b_lin2, in0=b_lin[:, 0], in1=b_lin[:, 1], op=ALU.add)

            # f2 combines
            nc.vector.tensor_tensor(out=shsq2, in0=sh2, in1=sh2, op=ALU.mult)
            nc.vector.tensor_tensor(out=s2, in0=sh2, in1=ch2, op=ALU.mult)
            nc.scalar.activation(out=t4, in_=s2, func=AF.Square, scale=1.0)
            nc.vector.tensor_scalar(out=c2, in0=shsq2, scalar1=-2.0, scalar2=1.0, op0=ALU.mult, op1=ALU.add)
            b_freq(1, s2, c2)

            # ladder gen1: F3 = 2f1, F4 = 2f2 (t3/t4 squares from ACT)
            nc.vector.tensor_tensor(out=s3, in0=s1, in1=c1, op=ALU.mult)
            nc.vector.tensor_scalar(out=c3_, in0=t3, scalar1=-8.0, scalar2=1.0, op0=ALU.mult, op1=ALU.add)
            b_freq(2, s3, c3_)
            nc.vector.tensor_tensor(out=s4, in0=s2, in1=c2, op=ALU.mult)
            nc.scalar.activation(out=t5, in_=s4, func=AF.Square, scale=1.0)
            nc.vector.tensor_scalar(out=c4, in0=t4, scalar1=-8.0, scalar2=1.0, op0=ALU.mult, op1=ALU.add)
            b_freq(3, s4, c4)

            # ladder gen2: F5 = 4f2 (t5 from ACT)
            nc.vector.tensor_tensor(out=s5, in0=s4, in1=c4, op=ALU.mult)
            nc.vector.tensor_scalar(out=c5, in0=t5, scalar1=-32.0, scalar2=1.0, op0=ALU.mult, op1=ALU.add)
            b_freq(4, s5, c5)

            # ---- content matmuls: one psum bank per t-block
            sp = spsum.tile([P, len(TBLK), 512], F32)
            mm_i = [0] * len(TBLK)
            N_MM = 25  # per t-block

            def emit_phase(entries, final=False):
                for tb, (t0, pn) in enumerate(TBLK):
                    sl = slice(t0, t0 + pn)
                    for ent in entries:
                        if ent[0] == "ones":
                            nc.tensor.matmul(
                                sp[:pn, tb, 0:U], lhsT=ones_a[:, :pn], rhs=ent[1],
                                start=(mm_i[tb] == 0), stop=(mm_i[tb] == N_MM - 1),
                            )
                            mm_i[tb] += 1
                        else:
                            a_fn, b_fn = ent
                            for m in range(HT):
                                nc.tensor.matmul(
                                    sp[:pn, tb, 0:U], lhsT=a_fn(m, sl), rhs=b_fn(m),
                                    start=(mm_i[tb] == 0), stop=(mm_i[tb] == N_MM - 1),
                                )
                                mm_i[tb] += 1
                    if final:
                        expt = soft.tile([P, U], F32, name=f"expt{tb}", bufs=2)
                        nc.scalar.activation(out=expt[:pn], in_=sp[:pn, tb, 0:U], func=AF.Exp, scale=1.0)
                        ssum = soft.tile([P, 1], F32, name=f"ssum{tb}", bufs=2)
                        nc.vector.tensor_reduce(out=ssum[:pn], in_=expt[:pn], axis=mybir.AxisListType.X, op=ALU.add)
                        nc.vector.reciprocal(out=ssum[:pn], in_=ssum[:pn])
                        outt = soft.tile([P, U], F32, name=f"outt{tb}", bufs=2)
                        nc.vector.tensor_scalar_mul(out=outt[:pn], in0=expt[:pn], scalar1=ssum[:pn])
                        nc.sync.dma_start(out=out_x[t0:t0 + pn, :], in_=outt[:pn])

            def freq_entries(i, s_t, c_t):
                return [
                    (lambda m, s, t=s_t: t[:, m, s], lambda m, i=i: b_s[i][:, 1, m]),
                    (lambda m, s, t=c_t: t[:, m, s], lambda m, i=i: b_s[i][:, 0, m]),
                ]

            emit_phase(freq_entries(0, s1, c1))           # f1 (ready first)
            emit_phase([                                   # poly
                (lambda m, s: epdp[:, m, s], lambda m: b_x[:, m]),
                (lambda m, s: sq[:, m, s], lambda m: b_x2[:, m]),
                ("ones", b_lin2),
            ])
            emit_phase(freq_entries(1, s2, c2))           # f2
            emit_phase(freq_entries(2, s3, c3_) + freq_entries(3, s4, c4))  # F3+F4
            emit_phase(freq_entries(4, s5, c5), final=True)  # F5 + softmax

    nc.finalize()
    return nc


_NC_CACHE = None


def kernel(**inputs: np.ndarray) -> np.ndarray:
    global _NC_CACHE
    bfd = ml_dtypes.bfloat16
    enc = np.asarray(inputs["encoder_out"], dtype=np.float32)
    dec = np.asarray(inputs["decoder_out"], dtype=np.float32)
    w_enc = np.asarray(inputs["W_enc"], np.float32)
    b_enc = np.asarray(inputs["b_enc"], dtype=np.float32)
    w_dec = np.asarray(inputs["W_dec"], np.float32)
    b_dec = np.asarray(inputs["b_dec"], dtype=np.float32)
    w_score = np.asarray(inputs["w_score"], dtype=np.float32)
    # b_score dropped: softmax(x + c) == softmax(x)

    def part_major(a2d):
        # [D, X] -> [P, KT*X]: partition p holds rows {k*P+p} k-major, contiguous
        X = a2d.shape[1]
        return np.ascontiguousarray(
            a2d.reshape(KT, P, X).transpose(1, 0, 2).reshape(P, KT * X).astype(bfd)
        )

    wcat = part_major(np.concatenate([w_enc, w_dec], axis=1))
    dec_pm = [part_major(dec[b].T) for b in range(B)]
    bias2 = np.ascontiguousarray((b_enc + b_dec).reshape(HT, P).T)

    # w_bc[p, slot, m, u]: 0..4 freq coefs cf*2^g*w, 5: 3c3*w, 6: c1*w, 7: c3*w
    wm = w_score.reshape(HT, P).T  # [P, m]
    w_bc = np.empty((P, NSLOT, HT, U), dtype=np.float32)
    for i in range(5):
        w_bc[:, i] = (CF[i] * (2 ** GENS[i]) * wm)[:, :, None]
    w_bc[:, 5] = (3 * C3 * wm)[:, :, None]
    w_bc[:, 6] = (C1 * wm)[:, :, None]
    w_bc[:, 7] = (C3 * wm)[:, :, None]
    w_bc = np.ascontiguousarray(w_bc.reshape(P, NSLOT * HT * U).astype(bfd))

    in_maps = []
    for c in range(NCORES):
        b = c // (NCORES // B)
        t0 = (c % (NCORES // B)) * TPC
        in_maps.append(
            {
                "enc_pm": part_major(enc[b, t0:t0 + TPC, :].T),
                "dec_pm": dec_pm[b],
                "wcat_pm": wcat,
                "w_bc": w_bc,
                "bias2": bias2,
            }
        )

    if _NC_CACHE is None:
        _NC_CACHE = _build_graph()
    res = run_bass_kernel_spmd(_NC_CACHE, in_maps, core_ids=list(range(NCORES)))

    out = np.empty((B, T, U), dtype=np.float32)
    for c in range(NCORES):
        b = c // (NCORES // B)
        t0 = (c % (NCORES // B)) * TPC
        out[b, t0:t0 + TPC, :] = res.results[c]["out"]
    return out


# revision 9
# speedup vs baseline: 1.2457x; 1.0088x over previous
"""Distributed Trainium2 kernel for the additive-attention alignment predictor.

Math: score[b,t,u] = sum_h w_h * tanh(ep[b,t,h] + dp[b,u,h]);  out = softmax_u(score)
  where ep = enc @ W_enc (bias folded into dp), dp = dec @ W_dec + b_enc + b_dec.
  (b_score dropped: softmax is shift-invariant; pure-x terms likewise dropped.)

tanh(z) on |z|<=6 is replaced by the separable expansion
  tanh(z) ~= c1*z + c3*z^3 + sum_f cf sin(w_f z)
with 5 frequencies built from 2 half-angle base evaluations (f1, f2) plus a
double-angle ladder (2*f1, 2*f2, 4*f2).  sin(w(x+y)) = sin cos + cos sin and
the cubic expand into 13 rank-1 products, so the whole [T,U,H] contraction
becomes TensorEngine matmuls.  Sin planes are stored as sin/2^g; the 2^g
factors fold into per-partition coefficient vectors (wq tile) that scale the
y-side planes via broadcast_to.

Scheduling: f2's sins are evaluated first because its ladder (2f2, 4f2) is the
longest dependency chain; f1 only feeds 2f1.  Ladder squares for the critical
chain run on DVE; f1's run on ACT's idle tail.  Inputs are partition-major,
per-k-slice descriptors spread over both HWDGE queues (weights on sync,
enc/dec on scalar; only late-needed tiles ride the slow gpsimd SWDGE ring).
Dummy ones-matmuls at kernel start keep the PE HAM clock warm through the
input DMA wait.

Sharding: data-parallel over (B, T/2): core c handles batch c//2, t-half c%2.
No cross-core communication.
"""

import math

import numpy as np
import ml_dtypes

import concourse.bass as bass
import concourse.tile as tile
from concourse import bacc, mybir
from concourse.bass_utils import run_bass_kernel_spmd

# Problem shapes (hardcoded per spec)
B, T, U = 4, 800, 150
D, H = 512, 256
NCORES = 8
TPC = T * B // NCORES  # 400 t-rows per core
P = 128
KT = D // P
HT = H // P
W550 = TPC + U  # 550
TBLK = [(i * P, min(P, TPC - i * P)) for i in range((TPC + P - 1) // P)]
N_WARMUP = 20

# Fitted expansion (fit_tanh2.py config C, coefs refit for these exact freqs):
#   tanh(z) ~= C1 z + C3 z^3 + sum cf sin(w z),  freqs {f1, f2, 2f1, 2f2, 4f2}
F1, F2 = 1.2150000011920929, 0.9102464900903208
FREQS = [F1, F2, 2 * F1, 2 * F2, 4 * F2]
C1, C3 = 0.40834739, -0.00686156
CF = [0.16673546, 0.1569309, 0.03624548, 0.06357671, 0.01070035]
GENS = [1, 1, 2, 2, 3]  # stored sin plane is sin(w z)/2^g

F32 = mybir.dt.float32
BF16 = mybir.dt.bfloat16
AF = mybir.ActivationFunctionType
ALU = mybir.AluOpType

# wq columns (per m): 0..4 freq coefs cf*2^g*w, 5: 3c3*w, 6: c1*w, 7: c3*w
NSLOT = 8


def _build_graph():
    nc = bacc.Bacc()
    # partition-major inputs: [P, k-major free] so DMA runs are contiguous
    enc_x = nc.declare_dram_parameter("enc_pm", [P, KT * TPC], BF16, isOutput=False)
    dec_x = nc.declare_dram_parameter("dec_pm", [P, KT * U], BF16, isOutput=False)
    we_x = nc.declare_dram_parameter("we_pm", [P, KT * H], BF16, isOutput=False)
    wd_x = nc.declare_dram_parameter("wd_pm", [P, KT * H], BF16, isOutput=False)
    wq_x = nc.declare_dram_parameter("wq", [P, NSLOT * HT], BF16, isOutput=False)
    bias_x = nc.declare_dram_parameter("bias2", [P, HT], F32, isOutput=False)
    out_x = nc.declare_dram_parameter("out", [TPC, U], F32, isOutput=True)

    with tile.TileContext(nc) as tc:
        with (
            tc.tile_pool(name="const", bufs=1) as const,
            tc.tile_pool(name="soft", bufs=1) as soft,
            tc.tile_pool(name="ppsum", bufs=1, space="PSUM") as ppsum,
            tc.tile_pool(name="spsum", bufs=1, space="PSUM") as spsum,
        ):
            # ---- input DMAs first: per-k-half descriptors, weights on sync
            enc_sb = const.tile([P, KT, TPC], BF16)
            dec_sb = const.tile([P, KT, U], BF16)
            we_sb = const.tile([P, KT, H], BF16)
            wd_sb = const.tile([P, KT, H], BF16)
            wq_sb = const.tile([P, NSLOT, HT], BF16)
            bias_sb = const.tile([P, HT], F32)
            EH = KT * TPC // 2
            WH = KT * H // 2
            nc.sync.dma_start(out=we_sb[:, 0:2, :], in_=we_x[:, 0:WH])
            nc.scalar.dma_start(out=enc_sb[:, 0:2, :], in_=enc_x[:, 0:EH])
            nc.sync.dma_start(out=wd_sb[:, 0:2, :], in_=wd_x[:, 0:WH])
            nc.scalar.dma_start(out=dec_sb, in_=dec_x[:, :])
            nc.sync.dma_start(out=we_sb[:, 2:4, :], in_=we_x[:, WH:])
            nc.scalar.dma_start(out=bias_sb, in_=bias_x[:])
            nc.sync.dma_start(out=wd_sb[:, 2:4, :], in_=wd_x[:, WH:])
            nc.scalar.dma_start(out=enc_sb[:, 2:4, :], in_=enc_x[:, EH:])
            nc.gpsimd.dma_start(out=wq_sb, in_=wq_x[:])

            # ---- constants
            ones_a = const.tile([P, P], BF16)
            nc.vector.memset(ones_a, 1.0)
            halfpi = const.tile([P, 1], F32)
            nc.vector.memset(halfpi, math.pi / 2)

            # ---- PE warm-up through the DMA wait
            ps_ep = [ppsum.tile([P, TPC], F32, name=f"ps_ep{m}") for m in range(HT)]
            ps_dp = [ppsum.tile([P, U], F32, name=f"ps_dp{m}") for m in range(HT)]
            for _ in range(N_WARMUP):
                nc.tensor.matmul(
                    ps_dp[0][:, 0:P], lhsT=ones_a, rhs=ones_a,
                    start=True, stop=True, skip_group_check=True,
                )

            # ---- projections (k-halves as the DMAs land)
            for k in range(KT):
                for m in range(HT):
                    nc.tensor.matmul(
                        ps_ep[m],
                        lhsT=we_sb[:, k, m * P:(m + 1) * P],
                        rhs=enc_sb[:, k, :],
                        start=(k == 0), stop=(k == KT - 1),
                    )
                for m in range(HT):
                    nc.tensor.matmul(
                        ps_dp[m],
                        lhsT=wd_sb[:, k, m * P:(m + 1) * P],
                        rhs=dec_sb[:, k, :],
                        start=(k == 0), stop=(k == KT - 1),
                    )

            # ---- epdp [P, m, 550]: x part 0:400, y part 400:550 (bf16)
            epdp = const.tile([P, HT, W550], BF16)
            for m in range(HT):
                nc.vector.tensor_scalar_add(
                    out=epdp[:, m, TPC:W550], in0=ps_dp[m],
                    scalar1=bias_sb[:, m:m + 1],
                )
                nc.vector.tensor_copy(epdp[:, m, 0:TPC], ps_ep[m])
            ep_y = epdp[:, :, TPC:W550]

            # ---- ACT: base Sin evals, f2 first (longest ladder chain)
            sh2 = const.tile([P, HT, W550], BF16)
            ch2 = const.tile([P, HT, W550], BF16)
            sh1 = const.tile([P, HT, W550], BF16)
            ch1 = const.tile([P, HT, W550], BF16)
            nc.scalar.activation(out=sh2, in_=epdp, func=AF.Sin, scale=float(F2 / 2))
            nc.scalar.activation(out=ch2, in_=epdp, func=AF.Sin, scale=float(F2 / 2), bias=halfpi[:, :])
            nc.scalar.activation(out=sh1, in_=epdp, func=AF.Sin, scale=float(F1 / 2))
            nc.scalar.activation(out=ch1, in_=epdp, func=AF.Sin, scale=float(F1 / 2), bias=halfpi[:, :])

            s1 = const.tile([P, HT, W550], BF16)   # sin(f1)/2
            c1 = const.tile([P, HT, W550], BF16)
            s2 = const.tile([P, HT, W550], BF16)   # sin(f2)/2
            c2 = const.tile([P, HT, W550], BF16)
            s3 = const.tile([P, HT, W550], BF16)   # sin(2f1)/4
            c3_ = const.tile([P, HT, W550], BF16)
            s4 = const.tile([P, HT, W550], BF16)   # sin(2f2)/4
            c4 = const.tile([P, HT, W550], BF16)
            s5 = const.tile([P, HT, W550], BF16)   # sin(4f2)/8
            c5 = const.tile([P, HT, W550], BF16)
            shsq1 = const.tile([P, HT, W550], BF16)
            shsq2 = const.tile([P, HT, W550], BF16)
            t3 = const.tile([P, HT, W550], BF16)   # s1^2
            t4 = const.tile([P, HT, W550], BF16)   # s2^2
            t5 = const.tile([P, HT, W550], BF16)   # s4^2
            sq = const.tile([P, HT, W550], BF16)   # epdp^2

            b_s = [const.tile([P, 2, HT, U], BF16, name=f"b_f{i}") for i in range(5)]

            def b_freq(i, s_t, c_t):
                wv = wq_sb[:, i, :].broadcast_to([P, HT, U])
                nc.vector.tensor_tensor(out=b_s[i][:, 0], in0=s_t[:, :, TPC:W550], in1=wv, op=ALU.mult)
                nc.vector.tensor_tensor(out=b_s[i][:, 1], in0=c_t[:, :, TPC:W550], in1=wv, op=ALU.mult)

            # ---- DVE: critical f2 ladder chain first (priority order)
            nc.vector.tensor_tensor(out=shsq2, in0=sh2, in1=sh2, op=ALU.mult)
            nc.vector.tensor_tensor(out=s2, in0=sh2, in1=ch2, op=ALU.mult)
            nc.vector.tensor_scalar(out=c2, in0=shsq2, scalar1=-2.0, scalar2=1.0, op0=ALU.mult, op1=ALU.add)
            b_freq(1, s2, c2)
            nc.vector.tensor_tensor(out=t4, in0=s2, in1=s2, op=ALU.mult)
            nc.vector.tensor_tensor(out=s4, in0=s2, in1=c2, op=ALU.mult)
            nc.vector.tensor_scalar(out=c4, in0=t4, scalar1=-8.0, scalar2=1.0, op0=ALU.mult, op1=ALU.add)
            b_freq(3, s4, c4)
            nc.vector.tensor_tensor(out=t5, in0=s4, in1=s4, op=ALU.mult)
            nc.vector.tensor_tensor(out=s5, in0=s4, in1=c4, op=ALU.mult)
            nc.vector.tensor_scalar(out=c5, in0=t5, scalar1=-32.0, scalar2=1.0, op0=ALU.mult, op1=ALU.add)
            b_freq(4, s5, c5)

            # f1 chain: square on ACT's tail, rest DVE
            nc.vector.tensor_tensor(out=s1, in0=sh1, in1=ch1, op=ALU.mult)
            nc.scalar.activation(out=shsq1, in_=sh1, func=AF.Square, scale=1.0)
            nc.vector.tensor_scalar(out=c1, in0=shsq1, scalar1=-2.0, scalar2=1.0, op0=ALU.mult, op1=ALU.add)
            b_freq(0, s1, c1)
            nc.vector.tensor_tensor(out=s3, in0=s1, in1=c1, op=ALU.mult)
            nc.scalar.activation(out=t3, in_=s1, func=AF.Square, scale=1.0)
            nc.vector.tensor_scalar(out=c3_, in0=t3, scalar1=-8.0, scalar2=1.0, op0=ALU.mult, op1=ALU.add)
            b_freq(2, s3, c3_)

            # poly planes (ready early; scheduler slots them into DVE idle gaps)
            nc.vector.tensor_tensor(out=sq, in0=epdp, in1=epdp, op=ALU.mult)
            sq_y = sq[:, :, TPC:W550]
            v3 = const.tile([P, HT, U], BF16)
            nc.vector.tensor_tensor(out=v3, in0=ep_y, in1=sq_y, op=ALU.mult)
            wv5 = wq_sb[:, 5, :].broadcast_to([P, HT, U])
            b_x2 = const.tile([P, HT, U], BF16)   # (3c3 w) y
            nc.vector.tensor_tensor(out=b_x2, in0=ep_y, in1=wv5, op=ALU.mult)
            b_x = const.tile([P, HT, U], BF16)    # (y^2 + c1/(3c3)) (3c3 w)
            nc.vector.scalar_tensor_tensor(
                out=b_x, in0=sq_y, scalar=float(C1 / (3 * C3)), in1=wv5,
                op0=ALU.add, op1=ALU.mult,
            )
            b_l1 = const.tile([P, HT, U], BF16)   # (c1 w) y
            nc.vector.tensor_tensor(out=b_l1, in0=ep_y, in1=wq_sb[:, 6, :].broadcast_to([P, HT, U]), op=ALU.mult)
            b_l3 = const.tile([P, HT, U], BF16)   # (c3 w) y^3
            nc.vector.tensor_tensor(out=b_l3, in0=v3, in1=wq_sb[:, 7, :].broadcast_to([P, HT, U]), op=ALU.mult)
            b_lin = const.tile([P, HT, U], BF16)
            nc.vector.tensor_tensor(out=b_lin, in0=b_l1, in1=b_l3, op=ALU.add)
            b_lin2 = const.tile([P, U], BF16)     # m-sum for the ones-lhsT pair
            nc.vector.tensor_tensor(out=b_lin2, in0=b_lin[:, 0], in1=b_lin[:, 1], op=ALU.add)

            # ---- content matmuls: one psum bank per t-block
            sp = spsum.tile([P, len(TBLK), 512], F32)
            mm_i = [0] * len(TBLK)
            N_MM = 25  # per t-block

            def emit_phase(entries, final=False):
                for tb, (t0, pn) in enumerate(TBLK):
                    sl = slice(t0, t0 + pn)
                    for ent in entries:
                        if ent[0] == "ones":
                            nc.tensor.matmul(
                                sp[:pn, tb, 0:U], lhsT=ones_a[:, :pn], rhs=ent[1],
                                start=(mm_i[tb] == 0), stop=(mm_i[tb] == N_MM - 1),
                            )
                            mm_i[tb] += 1
                        else:
                            a_fn, b_fn = ent
                            for m in range(HT):
                                nc.tensor.matmul(
                                    sp[:pn, tb, 0:U], lhsT=a_fn(m, sl), rhs=b_fn(m),
                                    start=(mm_i[tb] == 0), stop=(mm_i[tb] == N_MM - 1),
                                )
                                mm_i[tb] += 1
                    if final:
                        expt = soft.tile([P, U], F32, name=f"expt{tb}", bufs=2)
                        nc.scalar.activation(out=expt[:pn], in_=sp[:pn, tb, 0:U], func=AF.Exp, scale=1.0)
                        ssum = soft.tile([P, 1], F32, name=f"ssum{tb}", bufs=2)
                        nc.vector.tensor_reduce(out=ssum[:pn], in_=expt[:pn], axis=mybir.AxisListType.X, op=ALU.add)
                        nc.vector.reciprocal(out=ssum[:pn], in_=ssum[:pn])
                        outt = soft.tile([P, U], F32, name=f"outt{tb}", bufs=2)
                        nc.vector.tensor_scalar_mul(out=outt[:pn], in0=expt[:pn], scalar1=ssum[:pn])
                        nc.sync.dma_start(out=out_x[t0:t0 + pn, :], in_=outt[:pn])

            def freq_entries(i, s_t, c_t):
                return [
                    (lambda m, s, t=s_t: t[:, m, s], lambda m, i=i: b_s[i][:, 1, m]),
                    (lambda m, s, t=c_t: t[:, m, s], lambda m, i=i: b_s[i][:, 0, m]),
                ]

            emit_phase([                                   # poly (ready first)
                (lambda m, s: epdp[:, m, s], lambda m: b_x[:, m]),
                (lambda m, s: sq[:, m, s], lambda m: b_x2[:, m]),
                ("ones", b_lin2),
            ])
            emit_phase(freq_entries(1, s2, c2))           # f2
            emit_phase(freq_entries(3, s4, c4))           # 2f2
            emit_phase(freq_entries(0, s1, c1))           # f1
            emit_phase(freq_entries(2, s3, c3_))          # 2f1
            emit_phase(freq_entries(4, s5, c5), final=True)  # 4f2 + softmax

    nc.finalize()
    return nc


_NC_CACHE = None


def kernel(**inputs: np.ndarray) -> np.ndarray:
    global _NC_CACHE
    bfd = ml_dtypes.bfloat16
    enc = np.asarray(inputs["encoder_out"], dtype=np.float32)
    dec = np.asarray(inputs["decoder_out"], dtype=np.float32)
    w_enc = np.asarray(inputs["W_enc"], np.float32)
    b_enc = np.asarray(inputs["b_enc"], dtype=np.float32)
    w_dec = np.asarray(inputs["W_dec"], np.float32)
    b_dec = np.asarray(inputs["b_dec"], dtype=np.float32)
    w_score = np.asarray(inputs["w_score"], dtype=np.float32)
    # b_score dropped: softmax(x + c) == softmax(x)

    def part_major(a2d):
        # [D, X] -> [P, KT*X]: partition p holds rows {k*P+p} k-major, contiguous
        X = a2d.shape[1]
        return np.ascontiguousarray(
            a2d.reshape(KT, P, X).transpose(1, 0, 2).reshape(P, KT * X).astype(bfd)
        )

    we_pm = part_major(w_enc)
    wd_pm = part_major(w_dec)
    dec_pm = [part_major(dec[b].T) for b in range(B)]
    bias2 = np.ascontiguousarray((b_enc + b_dec).reshape(HT, P).T)

    # wq[p, slot, m]: 0..4 freq coefs cf*2^g*w, 5: 3c3*w, 6: c1*w, 7: c3*w
    wm = w_score.reshape(HT, P).T  # [P, m]
    wq = np.empty((P, NSLOT, HT), dtype=np.float32)
    for i in range(5):
        wq[:, i] = CF[i] * (2 ** GENS[i]) * wm
    wq[:, 5] = 3 * C3 * wm
    wq[:, 6] = C1 * wm
    wq[:, 7] = C3 * wm
    wq = np.ascontiguousarray(wq.reshape(P, NSLOT * HT).astype(bfd))

    in_maps = []
    for c in range(NCORES):
        b = c // (NCORES // B)
        t0 = (c % (NCORES // B)) * TPC
        in_maps.append(
            {
                "enc_pm": part_major(enc[b, t0:t0 + TPC, :].T),
                "dec_pm": dec_pm[b],
                "we_pm": we_pm,
                "wd_pm": wd_pm,
                "wq": wq,
                "bias2": bias2,
            }
        )

    if _NC_CACHE is None:
        _NC_CACHE = _build_graph()
    res = run_bass_kernel_spmd(_NC_CACHE, in_maps, core_ids=list(range(NCORES)))

    out = np.empty((B, T, U), dtype=np.float32)
    for c in range(NCORES):
        b = c // (NCORES // B)
        t0 = (c % (NCORES // B)) * TPC
        out[b, t0:t0 + TPC, :] = res.results[c]["out"]
    return out


# revision 11
# speedup vs baseline: 1.4207x; 1.1405x over previous
"""Distributed Trainium2 kernel for the additive-attention alignment predictor.

Math: score[b,t,u] = sum_h w_h * tanh(ep[b,t,h] + dp[b,u,h]);  out = softmax_u(score)
  where ep = enc @ W_enc (bias folded into dp), dp = dec @ W_dec + b_enc + b_dec.
  (b_score dropped: softmax is shift-invariant; pure-x terms likewise dropped.)

tanh(z) on |z|<=6 is replaced by the separable expansion
  tanh(z) ~= c1*z + c3*z^3 + cA sin(fA z) + cB sin(fB z) + cB2 sin(2 fB z)
(half-angle base evals for fA, fB; one double-angle step for 2fB).
sin(w(x+y)) = sin cos + cos sin and the cubic expand into 9 rank-1 products,
so the whole [T,U,H] contraction becomes TensorEngine matmuls.  Sin planes are
stored as sin/2^g; the 2^g factors fold into per-partition coefficient vectors
(wq tile) that scale the y-side planes via broadcast_to.

Engine split: ACT = dp-bias adds (Identity), 4 Sin evals (fB first: it owns
the ladder), shsqA square, softmax Exp (+accum row-sum); DVE = ep casts and
the sin/cos combine + B-plane chain; GpSimd = the slack-tolerant poly B-plane
chain.  Inputs are partition-major per-k-half descriptors over both HWDGE
queues.  Dummy ones-matmuls at kernel start keep the PE HAM clock warm
through the input DMA wait.

Sharding: data-parallel over (B, T/2): core c handles batch c//2, t-half c%2.
No cross-core communication.
"""

import math

import numpy as np
import ml_dtypes

import concourse.bass as bass
import concourse.tile as tile
from concourse import bacc, mybir
from concourse.bass_utils import run_bass_kernel_spmd

# Problem shapes (hardcoded per spec)
B, T, U = 4, 800, 150
D, H = 512, 256
NCORES = 8
TPC = T * B // NCORES  # 400 t-rows per core
P = 128
KT = D // P
HT = H // P
W550 = TPC + U  # 550
TB_W = 100
TBLK = [(i * TB_W, TB_W) for i in range(TPC // TB_W)]
N_WARMUP = 20

# Fitted expansion (config D): tanh(z) ~= C1 z + C3 z^3 + sum cf sin(w z)
FA, FB = 0.88, 1.215
FREQS = [FA, FB, 2 * FB]
C1, C3 = 0.49382319, -0.01153056
CF = [-0.08788495, 0.32848088, 0.06769629]
GENS = [1, 1, 2]  # stored sin plane is sin(w z)/2^g

F32 = mybir.dt.float32
BF16 = mybir.dt.bfloat16
AF = mybir.ActivationFunctionType
ALU = mybir.AluOpType

# wq columns (per m): 0..2 freq coefs cf*2^g*w, 3: 3c3*w, 4: c1*w, 5: c3*w
NSLOT = 6


def _build_graph():
    nc = bacc.Bacc()
    # partition-major inputs: [P, k-major free] so DMA runs are contiguous
    enc_x = nc.declare_dram_parameter("enc_pm", [P, KT * TPC], BF16, isOutput=False)
    dec_x = nc.declare_dram_parameter("dec_pm", [P, KT * U], BF16, isOutput=False)
    we_x = nc.declare_dram_parameter("we_pm", [P, KT * H], BF16, isOutput=False)
    wd_x = nc.declare_dram_parameter("wd_pm", [P, KT * H], BF16, isOutput=False)
    wq_x = nc.declare_dram_parameter("wq", [P, NSLOT * HT], BF16, isOutput=False)
    bias_x = nc.declare_dram_parameter("bias2", [P, HT], F32, isOutput=False)
    out_x = nc.declare_dram_parameter("out", [TPC, U], F32, isOutput=True)

    with tile.TileContext(nc) as tc:
        with (
            tc.tile_pool(name="const", bufs=1) as const,
            tc.tile_pool(name="soft", bufs=1) as soft,
            tc.tile_pool(name="ppsum", bufs=1, space="PSUM") as ppsum,
            tc.tile_pool(name="spsum", bufs=1, space="PSUM") as spsum,
        ):
            # ---- input DMAs first: per-k-half descriptors, weights on sync
            enc_sb = const.tile([P, KT, TPC], BF16)
            dec_sb = const.tile([P, KT, U], BF16)
            we_sb = const.tile([P, KT, H], BF16)
            wd_sb = const.tile([P, KT, H], BF16)
            wq_sb = const.tile([P, NSLOT, HT], BF16)
            bias_sb = const.tile([P, HT], F32)
            EH = KT * TPC // 2
            WH = KT * H // 2
            nc.sync.dma_start(out=we_sb[:, 0:2, :], in_=we_x[:, 0:WH])
            nc.scalar.dma_start(out=enc_sb[:, 0:2, :], in_=enc_x[:, 0:EH])
            nc.sync.dma_start(out=wd_sb[:, 0:2, :], in_=wd_x[:, 0:WH])
            nc.scalar.dma_start(out=dec_sb, in_=dec_x[:, :])
            nc.sync.dma_start(out=we_sb[:, 2:4, :], in_=we_x[:, WH:])
            nc.scalar.dma_start(out=bias_sb, in_=bias_x[:])
            nc.sync.dma_start(out=wd_sb[:, 2:4, :], in_=wd_x[:, WH:])
            nc.scalar.dma_start(out=enc_sb[:, 2:4, :], in_=enc_x[:, EH:])
            nc.gpsimd.dma_start(out=wq_sb, in_=wq_x[:])

            # ---- constants
            ones_a = const.tile([P, P], BF16)
            nc.vector.memset(ones_a, 1.0)
            halfpi = const.tile([P, 1], F32)
            nc.vector.memset(halfpi, math.pi / 2)

            # ---- PE warm-up through the DMA wait
            ps_ep = [ppsum.tile([P, TPC], F32, name=f"ps_ep{m}") for m in range(HT)]
            ps_dp = [ppsum.tile([P, U], F32, name=f"ps_dp{m}") for m in range(HT)]
            for _ in range(N_WARMUP):
                nc.tensor.matmul(
                    ps_dp[0][:, 0:P], lhsT=ones_a, rhs=ones_a,
                    start=True, stop=True, skip_group_check=True,
                )

            # ---- projections (k-halves as the DMAs land)
            for k in range(KT):
                for m in range(HT):
                    nc.tensor.matmul(
                        ps_ep[m],
                        lhsT=we_sb[:, k, m * P:(m + 1) * P],
                        rhs=enc_sb[:, k, :],
                        start=(k == 0), stop=(k == KT - 1),
                    )
                for m in range(HT):
                    nc.tensor.matmul(
                        ps_dp[m],
                        lhsT=wd_sb[:, k, m * P:(m + 1) * P],
                        rhs=dec_sb[:, k, :],
                        start=(k == 0), stop=(k == KT - 1),
                    )

            # ---- epdp [P, m, 550]: x 0:400 (DVE casts), y 400:550 (ACT adds)
            epdp = const.tile([P, HT, W550], BF16)
            for m in range(HT):
                nc.scalar.activation(
                    out=epdp[:, m, TPC:W550], in_=ps_dp[m], func=AF.Identity,
                    scale=1.0, bias=bias_sb[:, m:m + 1],
                )
                nc.vector.tensor_copy(epdp[:, m, 0:TPC], ps_ep[m])
            ep_y = epdp[:, :, TPC:W550]

            # ---- ACT: Sin evals, fB first (it owns the ladder)
            shB = const.tile([P, HT, W550], BF16)
            chB = const.tile([P, HT, W550], BF16)
            shA = const.tile([P, HT, W550], BF16)
            chA = const.tile([P, HT, W550], BF16)
            nc.scalar.activation(out=shB, in_=epdp, func=AF.Sin, scale=float(FB / 2))
            nc.scalar.activation(out=chB, in_=epdp, func=AF.Sin, scale=float(FB / 2), bias=halfpi[:, :])
            nc.scalar.activation(out=shA, in_=epdp, func=AF.Sin, scale=float(FA / 2))
            nc.scalar.activation(out=chA, in_=epdp, func=AF.Sin, scale=float(FA / 2), bias=halfpi[:, :])

            sA = const.tile([P, HT, W550], BF16)    # sin(fA)/2
            cA = const.tile([P, HT, W550], BF16)
            sB = const.tile([P, HT, W550], BF16)    # sin(fB)/2
            cB = const.tile([P, HT, W550], BF16)
            s2B = const.tile([P, HT, W550], BF16)   # sin(2fB)/4
            c2B = const.tile([P, HT, W550], BF16)
            shsqA = const.tile([P, HT, W550], BF16)
            shsqB = const.tile([P, HT, W550], BF16)
            tB = const.tile([P, HT, W550], BF16)    # sB^2
            sq = const.tile([P, HT, W550], BF16)    # epdp^2

            b_s = [const.tile([P, 2, HT, U], BF16, name=f"b_f{i}") for i in range(3)]

            def b_freq(i, s_t, c_t, eng=nc.vector):
                wv = wq_sb[:, i, :].broadcast_to([P, HT, U])
                eng.tensor_tensor(out=b_s[i][:, 0], in0=s_t[:, :, TPC:W550], in1=wv, op=ALU.mult)
                eng.tensor_tensor(out=b_s[i][:, 1], in0=c_t[:, :, TPC:W550], in1=wv, op=ALU.mult)

            # ---- DVE: critical fB ladder chain first (priority order)
            nc.vector.tensor_tensor(out=shsqB, in0=shB, in1=shB, op=ALU.mult)
            nc.vector.tensor_tensor(out=sB, in0=shB, in1=chB, op=ALU.mult)
            nc.vector.tensor_scalar(out=cB, in0=shsqB, scalar1=-2.0, scalar2=1.0, op0=ALU.mult, op1=ALU.add)
            b_freq(1, sB, cB)
            nc.vector.tensor_tensor(out=tB, in0=sB, in1=sB, op=ALU.mult)
            nc.vector.tensor_tensor(out=s2B, in0=sB, in1=cB, op=ALU.mult)
            nc.vector.tensor_scalar(out=c2B, in0=tB, scalar1=-8.0, scalar2=1.0, op0=ALU.mult, op1=ALU.add)
            b_freq(2, s2B, c2B)

            # fA chain (shsqA square on ACT's tail)
            nc.vector.tensor_tensor(out=sA, in0=shA, in1=chA, op=ALU.mult)
            nc.scalar.activation(out=shsqA, in_=shA, func=AF.Square, scale=1.0)
            nc.vector.tensor_scalar(out=cA, in0=shsqA, scalar1=-2.0, scalar2=1.0, op0=ALU.mult, op1=ALU.add)
            b_freq(0, sA, cA)

            # epdp^2 fills the DVE idle gap before shB lands
            nc.vector.tensor_tensor(out=sq, in0=epdp, in1=epdp, op=ALU.mult)
            sq_y = sq[:, :, TPC:W550]

            # ---- GpSimd: slack-tolerant poly B-plane chain
            wv3 = wq_sb[:, 3, :].broadcast_to([P, HT, U])
            b_x2 = const.tile([P, HT, U], BF16)   # (3c3 w) y
            nc.gpsimd.tensor_tensor(out=b_x2, in0=ep_y, in1=wv3, op=ALU.mult)
            b_l1 = const.tile([P, HT, U], BF16)   # (c1 w) y
            nc.gpsimd.tensor_tensor(out=b_l1, in0=ep_y, in1=wq_sb[:, 4, :].broadcast_to([P, HT, U]), op=ALU.mult)
            v3 = const.tile([P, HT, U], BF16)     # y^3
            nc.gpsimd.tensor_tensor(out=v3, in0=ep_y, in1=sq_y, op=ALU.mult)
            b_x = const.tile([P, HT, U], BF16)    # (y^2 + c1/(3c3)) (3c3 w)
            nc.vector.scalar_tensor_tensor(
                out=b_x, in0=sq_y, scalar=float(C1 / (3 * C3)), in1=wv3,
                op0=ALU.add, op1=ALU.mult,
            )
            b_l3 = const.tile([P, HT, U], BF16)   # (c3 w) y^3
            nc.gpsimd.tensor_tensor(out=b_l3, in0=v3, in1=wq_sb[:, 5, :].broadcast_to([P, HT, U]), op=ALU.mult)
            b_lin = const.tile([P, HT, U], BF16)
            nc.gpsimd.tensor_tensor(out=b_lin, in0=b_l1, in1=b_l3, op=ALU.add)
            b_lin2 = const.tile([P, U], BF16)     # m-sum for the ones-lhsT pair
            nc.gpsimd.tensor_tensor(out=b_lin2, in0=b_lin[:, 0], in1=b_lin[:, 1], op=ALU.add)

            # ---- content matmuls: one psum bank per t-block
            sp = spsum.tile([P, len(TBLK), 512], F32)
            mm_i = [0] * len(TBLK)
            N_MM = 17  # per t-block: 8 m-paired entries + ones pair

            def emit_phase(entries, final=False):
                for tb, (t0, pn) in enumerate(TBLK):
                    sl = slice(t0, t0 + pn)
                    for ent in entries:
                        if ent[0] == "ones":
                            nc.tensor.matmul(
                                sp[:pn, tb, 0:U], lhsT=ones_a[:, :pn], rhs=ent[1],
                                start=(mm_i[tb] == 0), stop=(mm_i[tb] == N_MM - 1),
                            )
                            mm_i[tb] += 1
                        else:
                            a_fn, b_fn = ent
                            for m in range(HT):
                                nc.tensor.matmul(
                                    sp[:pn, tb, 0:U], lhsT=a_fn(m, sl), rhs=b_fn(m),
                                    start=(mm_i[tb] == 0), stop=(mm_i[tb] == N_MM - 1),
                                )
                                mm_i[tb] += 1
                    if final:
                        expt = soft.tile([P, U], F32, name=f"expt{tb}", bufs=2)
                        ssum = soft.tile([P, 1], F32, name=f"ssum{tb}", bufs=2)
                        nc.scalar.activation(
                            out=expt[:pn], in_=sp[:pn, tb, 0:U], func=AF.Exp,
                            scale=1.0, accum_out=ssum[:pn],
                        )
                        nc.vector.reciprocal(out=ssum[:pn], in_=ssum[:pn])
                        outt = soft.tile([P, U], F32, name=f"outt{tb}", bufs=2)
                        nc.vector.tensor_scalar_mul(out=outt[:pn], in0=expt[:pn], scalar1=ssum[:pn])
                        nc.sync.dma_start(out=out_x[t0:t0 + pn, :], in_=outt[:pn])

            def freq_entries(i, s_t, c_t):
                return [
                    (lambda m, s, t=s_t: t[:, m, s], lambda m, i=i: b_s[i][:, 1, m]),
                    (lambda m, s, t=c_t: t[:, m, s], lambda m, i=i: b_s[i][:, 0, m]),
                ]

            emit_phase(freq_entries(1, sB, cB))            # fB
            emit_phase([                                    # poly
                (lambda m, s: epdp[:, m, s], lambda m: b_x[:, m]),
                (lambda m, s: sq[:, m, s], lambda m: b_x2[:, m]),
                ("ones", b_lin2),
            ])
            emit_phase(freq_entries(2, s2B, c2B))          # 2fB
            emit_phase(freq_entries(0, sA, cA), final=True)  # fA + softmax

    nc.finalize()
    return nc


_NC_CACHE = None


def kernel(**inputs: np.ndarray) -> np.ndarray:
    global _NC_CACHE
    bfd = ml_dtypes.bfloat16
    enc = np.asarray(inputs["encoder_out"], dtype=np.float32)
    dec = np.asarray(inputs["decoder_out"], dtype=np.float32)
    w_enc = np.asarray(inputs["W_enc"], np.float32)
    b_enc = np.asarray(inputs["b_enc"], dtype=np.float32)
    w_dec = np.asarray(inputs["W_dec"], np.float32)
    b_dec = np.asarray(inputs["b_dec"], dtype=np.float32)
    w_score = np.asarray(inputs["w_score"], dtype=np.float32)
    # b_score dropped: softmax(x + c) == softmax(x)

    def part_major(a2d):
        # [D, X] -> [P, KT*X]: partition p holds rows {k*P+p} k-major, contiguous
        X = a2d.shape[1]
        return np.ascontiguousarray(
            a2d.reshape(KT, P, X).transpose(1, 0, 2).reshape(P, KT * X).astype(bfd)
        )

    we_pm = part_major(w_enc)
    wd_pm = part_major(w_dec)
    dec_pm = [part_major(dec[b].T) for b in range(B)]
    bias2 = np.ascontiguousarray((b_enc + b_dec).reshape(HT, P).T)

    # wq[p, slot, m]: 0..2 freq coefs cf*2^g*w, 3: 3c3*w, 4: c1*w, 5: c3*w
    wm = w_score.reshape(HT, P).T  # [P, m]
    wq = np.empty((P, NSLOT, HT), dtype=np.float32)
    for i in range(3):
        wq[:, i] = CF[i] * (2 ** GENS[i]) * wm
    wq[:, 3] = 3 * C3 * wm
    wq[:, 4] = C1 * wm
    wq[:, 5] = C3 * wm
    wq = np.ascontiguousarray(wq.reshape(P, NSLOT * HT).astype(bfd))

    in_maps = []
    for c in range(NCORES):
        b = c // (NCORES // B)
        t0 = (c % (NCORES // B)) * TPC
        in_maps.append(
            {
                "enc_pm": part_major(enc[b, t0:t0 + TPC, :].T),
                "dec_pm": dec_pm[b],
                "we_pm": we_pm,
                "wd_pm": wd_pm,
                "wq": wq,
                "bias2": bias2,
            }
        )

    if _NC_CACHE is None:
        _NC_CACHE = _build_graph()
    res = run_bass_kernel_spmd(_NC_CACHE, in_maps, core_ids=list(range(NCORES)))

    out = np.empty((B, T, U), dtype=np.float32)
    for c in range(NCORES):
        b = c // (NCORES // B)
        t0 = (c % (NCORES // B)) * TPC
        out[b, t0:t0 + TPC, :] = res.results[c]["out"]
    return out
